# revision 1
# baseline (speedup 1.0000x reference)
"""Varlen causal attention (flash_attn_varlen semantics) on 8 Trainium2 cores.

Sharding: 16 heads across 8 cores (2 heads/core, Ulysses-style head shard,
identity comms). Each core runs the same SPMD Bass program on its head slice.

Per head: blocked attention over 128-row q blocks. For each q block only the
k blocks inside the (causal x segment) mask are computed -- the block structure
is specialized on the host from cu_seqlens at trace time. S = Q^T K runs in
float32r; P = exp(S * scale) in bf16 (logits are O(5), so no max subtraction
is needed); a ones-column appended to V yields the softmax denominator from
the same PV matmul.
"""

import numpy as np

L = 4096
H = 16
D = 128
N_CORES = 8
H_PER_CORE = H // N_CORES
SCALE = 1.0 / float(np.sqrt(D))
QB = 128  # q/k block size


def _seg_starts(cu: np.ndarray) -> np.ndarray:
    """Per-token segment start, exactly mirroring the reference searchsorted."""
    tok = np.arange(L)
    seg = np.searchsorted(cu[1:-1], tok, side="right")
    starts = np.concatenate([[0], cu[1:-1]])
    return starts[seg]


def _chunk_sizes(w: int) -> list:
    """Split w (multiple of 128) into matmul chunks <=512, avoiding <256
    trailing chunks (float32r runs at 1/4 rate below 256 free dim)."""
    sizes = [512] * (w // 512)
    rem = w % 512
    if rem:
        sizes.append(rem)
    if len(sizes) > 1 and sizes[-1] < 256:
        sizes[-2:] = [sizes[-2] - 128, sizes[-1] + 128]
    return sizes


def _build(cu: np.ndarray):
    import concourse.mybir as mybir
    import concourse.tile as tile
    from concourse import bacc
    from concourse.masks import make_identity

    f32 = mybir.dt.float32
    f32r = mybir.dt.float32r
    bf16 = mybir.dt.bfloat16
    AF = mybir.ActivationFunctionType

    seg_start = _seg_starts(cu)
    n_qb = L // QB

    nc = bacc.Bacc("TRN2", target_bir_lowering=False, debug=False,
                   num_devices=N_CORES)
    q_d = nc.dram_tensor("q", [L, H_PER_CORE, D], f32, kind="ExternalInput")
    k_d = nc.dram_tensor("k", [L, H_PER_CORE, D], f32, kind="ExternalInput")
    v_d = nc.dram_tensor("v", [L, H_PER_CORE, D], f32, kind="ExternalInput")
    o_d = nc.dram_tensor("out", [L, H_PER_CORE, D], f32, kind="ExternalOutput")

    with tile.TileContext(nc) as tc:
        with (
            tc.tile_pool(name="consts", bufs=1) as consts,
            tc.tile_pool(name="big", bufs=2) as big,
            tc.tile_pool(name="io", bufs=6) as io,
            tc.tile_pool(name="psb", bufs=3) as psb,
            tc.tile_pool(name="t_ps", bufs=2, space="PSUM") as tr_ps_pool,
            tc.tile_pool(name="s_ps", bufs=2, space="PSUM") as s_ps_pool,
            tc.tile_pool(name="o_ps", bufs=2, space="PSUM") as o_ps_pool,
        ):
            ident = consts.tile([128, 128], f32)
            make_identity(nc, ident[:])
            ident_bf = consts.tile([128, 128], bf16)
            nc.vector.tensor_copy(ident_bf[:], ident[:])

            for h in range(H_PER_CORE):
                # ---- prep: load + transpose Q,K; load + cast V (+ones col)
                qt_sb = big.tile([128, L], f32r, tag="qt")
                kt_sb = big.tile([128, L], f32r, tag="kt")
                v_sb = big.tile([128, n_qb, 132], bf16, tag="v")
                for t in range(n_qb):
                    r = slice(t * QB, (t + 1) * QB)
                    q_t = io.tile([128, D], f32, tag="q_in")
                    nc.sync.dma_start(q_t[:], q_d[r, h, :])
                    tp = tr_ps_pool.tile([128, 128], f32, tag="t")
                    nc.tensor.transpose(tp[:], q_t[:], ident[:])
                    nc.vector.tensor_copy(qt_sb[:, r], tp[:])

                    k_t = io.tile([128, D], f32, tag="k_in")
                    nc.sync.dma_start(k_t[:], k_d[r, h, :])
                    tp2 = tr_ps_pool.tile([128, 128], f32, tag="t")
                    nc.tensor.transpose(tp2[:], k_t[:], ident[:])
                    nc.vector.tensor_copy(kt_sb[:, r], tp2[:])

                    v_t = io.tile([128, D], f32, tag="v_in")
                    nc.sync.dma_start(v_t[:], v_d[r, h, :])
                    nc.vector.memset(v_sb[:, t, 0:1], 1.0)
                    nc.vector.tensor_copy(v_sb[:, t, 1:129], v_t[:])

                # ---- main: per q block
                for i in range(n_qb):
                    q0 = i * QB
                    k_lo_b = int(seg_start[q0]) // QB
                    k_lo = k_lo_b * QB
                    w = (i + 1) * QB - k_lo

                    p_sb = psb.tile([128, L], bf16, tag="p")

                    # S = (Q^T)^T K^T, chunked; P = exp(S * scale)
                    c0 = 0
                    for cw in _chunk_sizes(w):
                        s_ps = s_ps_pool.tile([128, 512], f32)
                        nc.tensor.matmul(
                            s_ps[:, :cw],
                            qt_sb[:, q0:q0 + QB],
                            kt_sb[:, k_lo + c0:k_lo + c0 + cw],
                            start=True, stop=True,
                        )
                        nc.scalar.activation(p_sb[:, c0:c0 + cw], s_ps[:, :cw],
                                             AF.Exp, scale=SCALE)
                        c0 += cw

                    # segment-boundary masking: rows whose segment starts at
                    # b > k_lo must drop columns [k_lo, b). Zeroing those
                    # columns for all rows >= b works because later segments
                    # need a superset zeroed. Partition offsets must be
                    # 32-aligned, so row-conditional zeroing goes through
                    # affine_select (predicate on the partition index).
                    for b in sorted(set(int(s) for s in seg_start[q0:q0 + QB])):
                        ncols = b - k_lo
                        if ncols <= 0:
                            continue
                        rb = b - q0
                        if rb <= 0:
                            nc.vector.memset(p_sb[:, 0:ncols], 0.0)
                        else:
                            # keep row p iff p < rb  <=>  (rb-1-p) >= 0
                            nc.gpsimd.affine_select(
                                out=p_sb[:, 0:ncols], in_=p_sb[:, 0:ncols],
                                compare_op=mybir.AluOpType.is_ge, fill=0.0,
                                base=rb - 1, pattern=[[0, ncols]],
                                channel_multiplier=-1,
                            )

                    # causal triangle on the diagonal block
                    nc.gpsimd.affine_select(
                        out=p_sb[:, w - QB:w], in_=p_sb[:, w - QB:w],
                        compare_op=mybir.AluOpType.is_ge, fill=0.0,
                        base=0, pattern=[[-1, QB]], channel_multiplier=1,
                    )

                    # O[:, 0] = denom, O[:, 1:129] = P @ V
                    o_ps = o_ps_pool.tile([128, 129], f32)
                    for j in range(k_lo_b, i + 1):
                        pt_sb = io.tile([128, 128], bf16, tag="pt")
                        nc.scalar.dma_start(
                            pt_sb[:], p_sb[:, (j - k_lo_b) * QB:(j - k_lo_b + 1) * QB],
                            transpose=True)
                        nc.tensor.matmul(o_ps[:], pt_sb[:], v_sb[:, j, 0:129],
                                         start=(j == k_lo_b), stop=(j == i))

                    recip = io.tile([128, 1], f32, tag="recip")
                    nc.vector.reciprocal(recip[:], o_ps[:, 0:1])
                    o_sb = io.tile([128, D], f32, tag="o_out")
                    nc.vector.tensor_scalar_mul(o_sb[:], o_ps[:, 1:129], recip[:])
                    nc.sync.dma_start(o_d[i * QB:(i + 1) * QB, h, :], o_sb[:])

    nc.compile()
    return nc


def _run(query, key, value, cu_seqlens, trace=False, **spmd_kwargs):
    from concourse import bass_utils

    query = np.ascontiguousarray(np.asarray(query, dtype=np.float32))
    key = np.ascontiguousarray(np.asarray(key, dtype=np.float32))
    value = np.ascontiguousarray(np.asarray(value, dtype=np.float32))
    cu = np.asarray(cu_seqlens, dtype=np.int64)

    nc = _build(cu)
    in_maps = []
    for c in range(N_CORES):
        hs = slice(c * H_PER_CORE, (c + 1) * H_PER_CORE)
        in_maps.append({
            "q": np.ascontiguousarray(query[:, hs, :]),
            "k": np.ascontiguousarray(key[:, hs, :]),
            "v": np.ascontiguousarray(value[:, hs, :]),
        })
    res = bass_utils.run_bass_kernel_spmd(nc, in_maps, list(range(N_CORES)),
                                          trace=trace, **spmd_kwargs)
    out = np.empty((L, H, D), dtype=np.float32)
    for c in range(N_CORES):
        out[:, c * H_PER_CORE:(c + 1) * H_PER_CORE, :] = res.results[c]["out"]
    return out, res


def kernel(query, key, value, cu_seqlens):
    out, _ = _run(query, key, value, cu_seqlens)
    return out



# revision 3
# speedup vs baseline: 9.2304x; 9.2304x over previous
"""Varlen causal attention (flash_attn_varlen semantics) on 8 Trainium2 cores.

Sharding: 16 heads across 8 cores (2 heads/core, Ulysses-style head shard,
identity comms). Each core runs the same SPMD Bass program on its head slice.

Key design (v2, transpose-free inner loop): compute S^T = K @ Q^T instead of
S = Q @ K^T.  Then P^T = exp(S^T * scale) comes out of the activation engine
already in [k, q] layout, which is exactly the stationary-operand layout the
PV matmul needs (lhsT = P^T chunk, rhs = V block) -- so the per-block P
transposes (370 DMA transposes, ~40% of the baseline critical path) vanish.

Per head:
  prep: load Q,K,V; PE-transpose Q,K into [D, L] bf16; V + ones-col in bf16.
  main: for each 256-row q superblock, for each in-mask k block j:
        S^T tile = K^T_j^T @ Q^T  (bf16, PSUM f32), exp on ScalarE (bf16 out),
        causal/segment masking on GpSimd, then PV matmuls accumulate
        O[q, 0:130] per 128-q chunk over j (col 0 = softmax denominator from
        a ones column in V).  Finalize: reciprocal + scale on DVE, DMA out.
The (I, j) tile list, trimmed to the causal x segment block mask, is
specialized on the host from cu_seqlens at trace time.
"""

import numpy as np

L = 4096
H = 16
D = 128
N_CORES = 8
H_PER_CORE = H // N_CORES
SCALE = 1.0 / float(np.sqrt(D))
QB = 128          # q/k block size
SB = 2            # q blocks per superblock (256 q rows)
GROUP_UNITS = 8   # 128-col units per S^T PSUM group tile ([128,1024] f32)
BANK_UNITS = 4    # units per PSUM bank


def _seg_starts(cu: np.ndarray) -> np.ndarray:
    """Per-token segment start, exactly mirroring the reference searchsorted."""
    tok = np.arange(L)
    seg = np.searchsorted(cu[1:-1], tok, side="right")
    starts = np.concatenate([[0], cu[1:-1]])
    return starts[seg]


def _build_plan(cu: np.ndarray):
    """Host-side specialization of the block-sparse attention pattern.

    Returns a list (one entry per superblock I) of dicts:
      groups: list of groups; each group is a list of units
              (unit_off, j, i) -- 128-col unit for k-block j, q-block i
      masks:  list of ("tri"|"rows"|"zero", group_idx, unit_off, *args)
      pv:     {chunk i: [(group_idx, unit_off, j), ...]} ascending j
    """
    ss = _seg_starts(cu)
    n_qb = L // QB
    k_lo_b = [int(ss[i * QB]) // QB for i in range(n_qb)]
    bounds = [int(b) for b in cu[1:-1] if 0 < int(b) < L]

    plan = []
    for I in range(n_qb // SB):
        i0, i1 = SB * I, SB * I + SB - 1
        groups = [[]]
        masks = []
        pv = {i: [] for i in range(i0, i1 + 1)}
        cursor = 0
        for j in range(k_lo_b[i0], i1 + 1):
            qsb = max(i0, j)
            qeb = qsb
            for i in range(qsb, i1 + 1):
                if k_lo_b[i] <= j:
                    qeb = i + 1
                else:
                    break
            if qeb <= qsb:
                continue
            for i in range(qsb, qeb):
                # new group if the current one is full
                if cursor == GROUP_UNITS:
                    groups.append([])
                    cursor = 0
                g = len(groups) - 1
                u = cursor
                groups[g].append((u, j, i))
                cursor += 1
                pv[i].append((g, u, j))
                # --- masks for this [k-block j] x [q-block i] unit ---
                if i == j:
                    masks.append(("tri", g, u))
                q0u = i * QB
                for b in bounds:
                    if j * QB < b < (j + 1) * QB:
                        c0 = max(0, b - q0u)
                        rb = b - j * QB
                        if c0 < QB:
                            masks.append(("rows", g, u, c0, rb))
                    elif (j + 1) * QB <= b:
                        c0 = b - q0u
                        if 0 <= c0 < QB:
                            masks.append(("zero", g, u, c0))
        plan.append({"groups": groups, "masks": masks, "pv": pv,
                     "i0": i0, "n_chunks": i1 - i0 + 1})
    return plan


def _build(cu: np.ndarray):
    import concourse.mybir as mybir
    import concourse.tile as tile
    from concourse import bacc
    from concourse.masks import make_identity

    f32 = mybir.dt.float32
    bf16 = mybir.dt.bfloat16
    AF = mybir.ActivationFunctionType
    n_qb = L // QB
    plan = _build_plan(cu)

    nc = bacc.Bacc("TRN2", target_bir_lowering=False, debug=False,
                   num_devices=N_CORES)
    q_d = nc.dram_tensor("q", [L, H_PER_CORE, D], f32, kind="ExternalInput")
    k_d = nc.dram_tensor("k", [L, H_PER_CORE, D], f32, kind="ExternalInput")
    v_d = nc.dram_tensor("v", [L, H_PER_CORE, D], f32, kind="ExternalInput")
    o_d = nc.dram_tensor("out", [L, H_PER_CORE, D], f32, kind="ExternalOutput")

    with tile.TileContext(nc) as tc:
        with (
            tc.tile_pool(name="consts", bufs=1) as consts,
            tc.tile_pool(name="stage", bufs=2) as stage,
            tc.tile_pool(name="big", bufs=2) as big,
            tc.tile_pool(name="psb", bufs=10) as psb,
            tc.tile_pool(name="osb", bufs=2) as osb,
            tc.tile_pool(name="rsb", bufs=2) as rsb,
            tc.tile_pool(name="s_ps", bufs=2, space="PSUM") as s_ps,
            tc.tile_pool(name="o_ps", bufs=2, space="PSUM") as o_ps,
            tc.tile_pool(name="tr_ps", bufs=2, space="PSUM") as tr_ps,
        ):
            ident = consts.tile([128, 128], f32)
            make_identity(nc, ident[:])

            # ---- upfront DMA loads for both heads (keeps load issue ahead
            # of store issue on the sync queue) ----
            stages = {}
            for h in range(H_PER_CORE):
                qs = stage.tile([128, n_qb, D], f32, tag="qs")
                ks = stage.tile([128, n_qb, D], f32, tag="ks")
                vs = stage.tile([128, n_qb, D], f32, tag="vs")
                for b0 in range(0, n_qb, 8):
                    r = slice(b0 * QB, (b0 + 8) * QB)
                    nc.sync.dma_start(
                        qs[:, b0:b0 + 8, :],
                        q_d[r, h, :].rearrange("(t p) d -> p t d", p=128))
                    nc.sync.dma_start(
                        ks[:, b0:b0 + 8, :],
                        k_d[r, h, :].rearrange("(t p) d -> p t d", p=128))
                    nc.sync.dma_start(
                        vs[:, b0:b0 + 8, :],
                        v_d[r, h, :].rearrange("(t p) d -> p t d", p=128))
                stages[h] = (qs, ks, vs)

            for h in range(H_PER_CORE):
                qs, ks, vs = stages[h]
                # ---- prep: V (+ones col) in bf16; Q,K transposed to [D, L]
                vA = big.tile([128, n_qb, 130], bf16, tag="vA")
                nc.gpsimd.memset(vA[:, :, 0:1], 1.0)
                for b0 in range(0, n_qb, 8):
                    nc.gpsimd.tensor_copy(vA[:, b0:b0 + 8, 1:129],
                                          vs[:, b0:b0 + 8, :])
                qT = big.tile([128, L], bf16, tag="qT")
                kT = big.tile([128, L], bf16, tag="kT")
                for src, dstT in ((qs, qT), (ks, kT)):
                    for b0 in range(0, n_qb, 4):
                        trp = tr_ps.tile([128, 4, 128], f32, tag="tr")
                        for t in range(4):
                            nc.tensor.transpose(trp[:, t, :],
                                                src[:, b0 + t, :], ident[:])
                        nc.vector.tensor_copy(
                            dstT[:, b0 * QB:(b0 + 4) * QB], trp[:, :, :])

                # ---- main loop, software-pipelined by one superblock:
                # emit S^T+exp+masks for I, then PV+finalize for I-1.
                pending = None

                def emit_pv_finalize(pend):
                    I, ptiles = pend
                    sbp = plan[I]
                    i0 = sbp["i0"]
                    o_t = o_ps.tile([128, 512], f32, tag="o")
                    for c in range(sbp["n_chunks"]):
                        i = i0 + c
                        lst = sbp["pv"][i]
                        for n, (g, u, j) in enumerate(lst):
                            nc.tensor.matmul(
                                o_t[:, c * 130:c * 130 + 130],
                                ptiles[g][:, u * QB:(u + 1) * QB],
                                vA[:, j, 0:130],
                                start=(n == 0), stop=(n == len(lst) - 1))
                    rec = rsb.tile([128, 2, 1], f32, tag="r")
                    den = o_t[:, 0:260].rearrange("p (c x) -> p c x", c=2)
                    nc.vector.reciprocal(rec[:, :, :], den[:, :, 0:1])
                    ost = osb.tile([128, 2, 128], f32, tag="ost")
                    for c in range(sbp["n_chunks"]):
                        nc.vector.tensor_scalar_mul(
                            ost[:, c, :], o_t[:, c * 130 + 1:c * 130 + 129],
                            rec[:, c, :])
                    nc.sync.dma_start(
                        o_d[i0 * QB:(i0 + SB) * QB, h, :].rearrange(
                            "(t p) d -> p t d", p=128),
                        ost[:, :, :])

                for I, sbp in enumerate(plan):
                    ptiles = []
                    for g, group in enumerate(sbp["groups"]):
                        if not group:
                            continue
                        s_t = s_ps.tile([128, 1024], f32, tag="s")
                        p_t = psb.tile([128, 1024], bf16, tag="p")
                        # merge units with consecutive (j, i) into one matmul,
                        # not crossing bank boundaries
                        runs = []
                        for (u, j, i) in group:
                            if (runs and runs[-1][1] == j
                                    and runs[-1][2] + runs[-1][3] == i
                                    and runs[-1][0] + runs[-1][3] == u
                                    and (u % BANK_UNITS) != 0):
                                runs[-1][3] += 1
                            else:
                                runs.append([u, j, i, 1])
                        for (u, j, i, n) in runs:
                            nc.tensor.matmul(
                                s_t[:, u * QB:(u + n) * QB],
                                kT[:, j * QB:(j + 1) * QB],
                                qT[:, i * QB:(i + n) * QB],
                                start=True, stop=True)
                        gw = (group[-1][0] + 1) * QB
                        nc.scalar.activation(p_t[:, 0:gw], s_t[:, 0:gw],
                                             AF.Exp, scale=SCALE)
                        ptiles.append(p_t)
                    # masks (gpsimd), after exp
                    for m in sbp["masks"]:
                        kind, g, u = m[0], m[1], m[2]
                        p_t = ptiles[g]
                        sl = p_t[:, u * QB:(u + 1) * QB]
                        if kind == "tri":
                            # keep q >= k: iota = -p + c >= 0
                            nc.gpsimd.affine_select(
                                out=sl, in_=sl,
                                compare_op=mybir.AluOpType.is_ge, fill=0.0,
                                base=0, pattern=[[1, QB]],
                                channel_multiplier=-1)
                        elif kind == "rows":
                            c0, rb = m[3], m[4]
                            sl2 = p_t[:, u * QB + c0:(u + 1) * QB]
                            # keep k-rows >= rb: iota = p - rb >= 0
                            nc.gpsimd.affine_select(
                                out=sl2, in_=sl2,
                                compare_op=mybir.AluOpType.is_ge, fill=0.0,
                                base=-rb, pattern=[[0, QB - c0]],
                                channel_multiplier=1)
                        else:  # "zero"
                            c0 = m[3]
                            nc.gpsimd.memset(p_t[:, u * QB + c0:(u + 1) * QB],
                                             0.0)
                    if pending is not None:
                        emit_pv_finalize(pending)
                    pending = (I, ptiles)
                emit_pv_finalize(pending)
                pending = None

    nc.compile()
    return nc


def _run(query, key, value, cu_seqlens, trace=False, **spmd_kwargs):
    from concourse import bass_utils

    query = np.ascontiguousarray(np.asarray(query, dtype=np.float32))
    key = np.ascontiguousarray(np.asarray(key, dtype=np.float32))
    value = np.ascontiguousarray(np.asarray(value, dtype=np.float32))
    cu = np.asarray(cu_seqlens, dtype=np.int64)

    nc = _build(cu)
    in_maps = []
    for c in range(N_CORES):
        hs = slice(c * H_PER_CORE, (c + 1) * H_PER_CORE)
        in_maps.append({
            "q": np.ascontiguousarray(query[:, hs, :]),
            "k": np.ascontiguousarray(key[:, hs, :]),
            "v": np.ascontiguousarray(value[:, hs, :]),
        })
    res = bass_utils.run_bass_kernel_spmd(nc, in_maps, list(range(N_CORES)),
                                          trace=trace, **spmd_kwargs)
    out = np.empty((L, H, D), dtype=np.float32)
    for c in range(N_CORES):
        out[:, c * H_PER_CORE:(c + 1) * H_PER_CORE, :] = res.results[c]["out"]
    return out, res


def kernel(query, key, value, cu_seqlens):
    out, _ = _run(query, key, value, cu_seqlens)
    return out
